# revision 9
# baseline (speedup 1.0000x reference)
"""TRN2 Bass kernel for nn_Attention_79620103733970.

Causal multi-head attention (GPT-style attention block):
  q/k/v = x @ w{q,k,v} + b, RoPE on q/k, causal softmax attention,
  y = att_out @ wc + bc.

Sharding: pure tensor-parallel over heads across 8 NeuronCores
(H=16 heads -> 2 heads/core, full batch per core). After attention each
core holds O^T for its 2 heads over all B*T rows; one 8-way AllToAll
redistributes so core j owns all 16 heads for global row chunk j,
where it computes the output projection + bias.

Per-core on-device layouts:
  xT      [C=1024, B*T]      8 tiles [128, B*T] bf16 (DMA transpose)
  qT/kT   [128, B*T] bf16    2 heads x 64 dims on partitions, RoPE'd
  v       [128, rt, 130] bf16  rows on partitions; per head 64 cols + ones
  S^T     [k=128, q<=512] f32 PSUM = K @ Q^T per k-block (2 heads packed
                                     on PE row groups 0-63 / 64-127)
  P^T     exp(S^T/8) bf16 SBUF (ScalarE; causal ranges + tril mask)
  O_aug   [65, 512] f32 PSUM = v_aug^T @ P^T over k-blocks; row 64 = denom
  OT      [128, B*T] bf16    normalized attention output^T
  y       [B*T/8, 1024] f32  output rows for this core's global chunk
"""

import sys

sys.path.insert(0, "/opt/trn_rl_repo")

import numpy as np
import ml_dtypes

import concourse.bacc as bacc
import concourse.mybir as mybir
from concourse.tile import TileContext

F32 = mybir.dt.float32
BF16 = mybir.dt.bfloat16
Alu = mybir.AluOpType
Act = mybir.ActivationFunctionType

N_CORES = 8
B, T, C = 2, 2048, 1024
H, HS = 16, 64
THETA = 10000.0
QCH = 512                   # q chunk (columns per attention chunk)
KB = 128                    # k block rows
VW = 2 * (HS + 1)           # v width per row-tile: 2 heads * (64 + ones col)

_CACHE = {}


def _build(t, debug=False):
    """Build the SPMD kernel module for per-batch sequence length t."""
    rows = B * t               # global rows
    n_rt = rows // 128         # v row tiles
    n_ch = t // QCH            # attention chunks per batch
    SH = rows // N_CORES       # a2a shard columns = output rows per core
    n_yrt = SH // 128

    nc = bacc.Bacc(None, num_devices=N_CORES)

    # ---------------- DRAM I/O ----------------
    xb = nc.dram_tensor("xb", [rows, C], BF16, kind="ExternalInput")
    wq = nc.dram_tensor("wq", [C, 128], BF16, kind="ExternalInput")
    wk = nc.dram_tensor("wk", [C, 128], BF16, kind="ExternalInput")
    wv = nc.dram_tensor("wv", [C, 128], BF16, kind="ExternalInput")
    wc = nc.dram_tensor("wc", [C, C], BF16, kind="ExternalInput")
    bq = nc.dram_tensor("bq", [128], F32, kind="ExternalInput")
    bk = nc.dram_tensor("bk", [128], F32, kind="ExternalInput")
    bqs = nc.dram_tensor("bqs", [128], F32, kind="ExternalInput")
    bks = nc.dram_tensor("bks", [128], F32, kind="ExternalInput")
    bvp = nc.dram_tensor("bvp", [VW], F32, kind="ExternalInput")
    bcb = nc.dram_tensor("bcb", [C], BF16, kind="ExternalInput")
    cosd = nc.dram_tensor("cosd", [128, t], BF16, kind="ExternalInput")
    sind = nc.dram_tensor("sind", [128, t], BF16, kind="ExternalInput")
    trild = nc.dram_tensor("trild", [128, 128], BF16, kind="ExternalInput")

    y_ext = nc.dram_tensor("y", [SH, C], F32, kind="ExternalOutput")

    if debug:
        dbg_qT = nc.dram_tensor("dbg_qT", [128, rows], BF16, kind="ExternalOutput")
        dbg_kT = nc.dram_tensor("dbg_kT", [128, rows], BF16, kind="ExternalOutput")
        dbg_v = nc.dram_tensor("dbg_v", [128, rows // 128, VW], BF16, kind="ExternalOutput")
        dbg_OT = nc.dram_tensor("dbg_OT", [128, rows], BF16, kind="ExternalOutput")
        dbg_P = nc.dram_tensor("dbg_P", [4, 128, 1024], BF16, kind="ExternalOutput")
        dbg_O = nc.dram_tensor("dbg_O", [65, 512], F32, kind="ExternalOutput")

    a2a_in = nc.dram_tensor("a2a_in", [N_CORES, 128, SH], BF16)
    a2a_out = nc.dram_tensor("a2a_out", [N_CORES, 128, SH], BF16)

    with TileContext(nc) as tc:
        with tc.tile_pool(name="persist", bufs=1) as pp, \
             tc.tile_pool(name="qkv", bufs=1) as qkvp, \
             tc.tile_pool(name="ptiles", bufs=6) as ppool, \
             tc.tile_pool(name="small", bufs=4) as smallp, \
             tc.tile_pool(name="misc", bufs=2) as miscp:

            # ---------- constants ----------
            cos_sb = pp.tile([128, t], BF16, name="cos_sb")
            sin_sb = pp.tile([128, t], BF16, name="sin_sb")
            nc.sync.dma_start(out=cos_sb[:], in_=cosd[:])
            nc.sync.dma_start(out=sin_sb[:], in_=sind[:])
            tril_sb = pp.tile([128, 128], BF16, name="tril_sb")
            nc.sync.dma_start(out=tril_sb[:], in_=trild[:])
            wc_sb = pp.tile([128, 8, C], BF16, name="wc_sb")
            nc.sync.dma_start(
                out=wc_sb[:], in_=wc[:].rearrange("(kt p) n -> p kt n", p=128))
            bc_sb = pp.tile([1, C], BF16, name="bc_sb")
            nc.sync.dma_start(out=bc_sb[:], in_=bcb[:].unsqueeze(0))
            ones_bf = pp.tile([1, 128], BF16, name="ones_bf")
            nc.vector.memset(ones_bf[:], 1.0)
            ones_f32 = pp.tile([1, 64], F32, name="ones_f32")
            nc.vector.memset(ones_f32[:], 1.0)
            bq_sb = pp.tile([128, 1], F32, name="bq_sb")
            bk_sb = pp.tile([128, 1], F32, name="bk_sb")
            bqs_sb = pp.tile([128, 1], F32, name="bqs_sb")
            bks_sb = pp.tile([128, 1], F32, name="bks_sb")
            nc.sync.dma_start(out=bq_sb[:], in_=bq[:].unsqueeze(1))
            nc.sync.dma_start(out=bk_sb[:], in_=bk[:].unsqueeze(1))
            nc.sync.dma_start(out=bqs_sb[:], in_=bqs[:].unsqueeze(1))
            nc.sync.dma_start(out=bks_sb[:], in_=bks[:].unsqueeze(1))
            bv_sb = pp.tile([128, VW], F32, name="bv_sb")
            nc.sync.dma_start(
                out=bv_sb[:], in_=bvp[:].unsqueeze(0).broadcast_to([128, VW]))

            # ---------- weights ----------
            wq_sb = pp.tile([128, 8, 128], BF16, name="wq_sb")
            wk_sb = pp.tile([128, 8, 128], BF16, name="wk_sb")
            wv_sb = pp.tile([128, 8, 128], BF16, name="wv_sb")
            for wsb, wdr in ((wq_sb, wq), (wk_sb, wk), (wv_sb, wv)):
                nc.sync.dma_start(
                    out=wsb[:], in_=wdr[:].rearrange("(kt p) m -> p kt m", p=128))

            # ---------- qkv destination tiles ----------
            qT = qkvp.tile([128, rows], BF16, name="qT")
            kT = qkvp.tile([128, rows], BF16, name="kT")
            v_sb = qkvp.tile([128, n_rt, VW], BF16, name="v_sb")
            nc.vector.memset(v_sb[:], 1.0)  # ones cols; data cols overwritten
            OT = qkvp.tile([128, rows], BF16, name="OT")

            # ================= phase 1: xT + projections + rope ==========
            with tc.tile_pool(name="xt", bufs=1) as xtp, \
                 tc.tile_pool(name="rope", bufs=1) as ropep, \
                 tc.tile_pool(name="pjps", bufs=4, space="PSUM") as pjps, \
                 tc.tile_pool(name="vps", bufs=4, space="PSUM") as vps:

                xT = xtp.tile([128, 8, rows], BF16, name="xT")
                for ct in range(8):
                    nc.sync.dma_start(out=xT[:, ct, :],
                                      in_=xb[:, 128 * ct:128 * ct + 128],
                                      transpose=True)

                def project_T(w_sb, dst_raw):
                    # dst_raw[m, g] = sum_c w[c, m] * xT[c, g]
                    for nb in range(rows // 512):
                        ps = pjps.tile([128, 512], F32, name="pj")
                        for kt in range(8):
                            nc.tensor.matmul(
                                ps[:], w_sb[:, kt, :],
                                xT[:, kt, 512 * nb:512 * nb + 512],
                                start=(kt == 0), stop=(kt == 7))
                        nc.any.tensor_copy(
                            dst_raw[:, 512 * nb:512 * nb + 512], ps[:])

                q_raw = ropep.tile([128, rows], BF16, name="q_raw")
                k_raw = ropep.tile([128, rows], BF16, name="k_raw")
                project_T(wq_sb, q_raw)
                project_T(wk_sb, k_raw)

                # v natural layout [row, head-dim]
                for rt in range(n_rt):
                    ps = vps.tile([128, 128], F32, name="vps")
                    for kt in range(8):
                        nc.tensor.matmul(
                            ps[:], xT[:, kt, 128 * rt:128 * rt + 128],
                            wv_sb[:, kt, :],
                            start=(kt == 0), stop=(kt == 7))
                    dst = v_sb[:, rt, :].rearrange(
                        "p (h e) -> p h e", h=2)[:, :, 0:HS]
                    src = ps[:].rearrange("p (h d) -> p h d", h=2)
                    bvv = bv_sb[:].rearrange(
                        "p (h e) -> p h e", h=2)[:, :, 0:HS]
                    nc.vector.scalar_tensor_tensor(
                        dst, src, 1.0, bvv, Alu.mult, Alu.add)

                # ---------- RoPE ----------
                def rope(raw, bias, bias_s, dst):
                    shf = ropep.tile([128, rows], BF16, name="shf")
                    for g in range(4):
                        s, d = 32 * (g ^ 1), 32 * g
                        nc.sync.dma_start(out=shf[d:d + 32, :],
                                          in_=raw[s:s + 32, :])
                    tmp = ropep.tile([128, rows], BF16, name="tmp")
                    tmp2 = ropep.tile([128, rows], BF16, name="tmp2")
                    for bb in range(B):
                        sl = slice(bb * t, bb * t + t)
                        nc.vector.scalar_tensor_tensor(
                            tmp[:, sl], raw[:, sl], bias[:], cos_sb[:],
                            Alu.add, Alu.mult)
                        nc.vector.scalar_tensor_tensor(
                            tmp2[:, sl], shf[:, sl], bias_s[:], sin_sb[:],
                            Alu.add, Alu.mult)
                        nc.vector.tensor_tensor(
                            dst[:, sl], tmp[:, sl], tmp2[:, sl], Alu.add)

                rope(q_raw, bq_sb, bqs_sb, qT)
                rope(k_raw, bk_sb, bks_sb, kT)

            # ================= phase 2: attention =================
            with tc.tile_pool(name="sbig", bufs=2, space="PSUM") as sbig, \
                 tc.tile_pool(name="ops", bufs=4, space="PSUM") as opsp:

                for b in range(B):
                    base = b * t
                    vbase = (t // 128) * b
                    for ch in range(n_ch):
                        qcol = base + QCH * ch
                        o_ps = [opsp.tile([128, 512], F32, name="o_ps")
                                for _ in range(2)]
                        nkb = 4 * (ch + 1)
                        # non-diagonal k-blocks, groups of 2
                        for kb0 in range(0, 4 * ch, 2):
                            pts = []
                            for h in range(2):
                                s_ps = sbig.tile([128, 1024], F32, name="sbig")
                                for i in range(2):
                                    kb = kb0 + i
                                    nc.tensor.matmul(
                                        s_ps[:, 512 * i:512 * i + 512],
                                        kT[64 * h:64 * h + 64,
                                           base + KB * kb:base + KB * kb + KB],
                                        qT[64 * h:64 * h + 64, qcol:qcol + QCH],
                                        start=True, stop=True,
                                        tile_position=(64 * h, 0))
                                pt = ppool.tile([128, 1024], BF16, name="pt")
                                nc.scalar.activation(pt[:], s_ps[:], Act.Exp,
                                                     scale=0.125)
                                pts.append(pt)
                            for h in range(2):
                                for i in range(2):
                                    kb = kb0 + i
                                    nc.tensor.matmul(
                                        o_ps[h][0:65, :],
                                        v_sb[:, vbase + kb,
                                             (HS + 1) * h:(HS + 1) * h + 65],
                                        pts[h][:, 512 * i:512 * i + 512],
                                        start=(kb == 0), stop=False)
                        # diagonal k-blocks (rel 0..3), causal-restricted
                        for rel in range(4):
                            kb = 4 * ch + rel
                            n_valid = QCH - KB * rel
                            pts = []
                            for h in range(2):
                                s_ps = sbig.tile([128, 1024], F32, name="sbig")
                                nc.tensor.matmul(
                                    s_ps[:, 0:n_valid],
                                    kT[64 * h:64 * h + 64,
                                       base + KB * kb:base + KB * kb + KB],
                                    qT[64 * h:64 * h + 64,
                                       qcol + KB * rel:qcol + QCH],
                                    start=True, stop=True,
                                    tile_position=(64 * h, 0))
                                pt = ppool.tile([128, 1024], BF16, name="pt")
                                if n_valid > 128:
                                    nc.scalar.activation(
                                        pt[:, 128:n_valid],
                                        s_ps[:, 128:n_valid],
                                        Act.Exp, scale=0.125)
                                ptm = ppool.tile([128, 128], BF16, name="ptm")
                                nc.scalar.activation(ptm[:], s_ps[:, 0:128],
                                                     Act.Exp, scale=0.125)
                                nc.vector.tensor_tensor(
                                    pt[:, 0:128], ptm[:], tril_sb[:],
                                    Alu.mult)
                                if debug and b == 0 and ch == 0 and h == 0:
                                    nc.sync.dma_start(
                                        out=dbg_P[rel][:, 0:n_valid],
                                        in_=pt[:, 0:n_valid])
                                pts.append(pt)
                            for h in range(2):
                                nc.tensor.matmul(
                                    o_ps[h][0:65, KB * rel:QCH],
                                    v_sb[:, vbase + kb,
                                         (HS + 1) * h:(HS + 1) * h + 65],
                                    pts[h][:, 0:n_valid],
                                    start=(kb == 0), stop=(kb == nkb - 1))
                        # normalize -> OT[64h:64h+64, qcol:qcol+512]
                        if debug and b == 0 and ch == 0:
                            osb_d = miscp.tile([65, 512], F32, name="osb_d")
                            nc.vector.tensor_copy(osb_d[:], o_ps[0][0:65, :])
                            nc.sync.dma_start(out=dbg_O[:], in_=osb_d[:])
                        for h in range(2):
                            inv_r = smallp.tile([1, 512], F32, name="inv_r")
                            nc.vector.reciprocal(inv_r[:], o_ps[h][64:65, :])
                            bc_ps = sbig.tile([128, 1024], F32, name="sbig")
                            nc.tensor.matmul(bc_ps[0:64, 0:512], ones_f32[:],
                                             inv_r[:], start=True, stop=True)
                            bcs = miscp.tile([64, 512], F32, name="bcs")
                            nc.any.tensor_copy(bcs[:], bc_ps[0:64, 0:512])
                            nc.vector.tensor_tensor(
                                OT[64 * h:64 * h + 64, qcol:qcol + QCH],
                                o_ps[h][0:64, :], bcs[:], Alu.mult)

            # ================= phase 3: a2a + c_proj =================
            if debug:
                nc.sync.dma_start(out=dbg_qT[:], in_=qT[:])
                nc.sync.dma_start(out=dbg_kT[:], in_=kT[:])
                nc.sync.dma_start(out=dbg_v[:], in_=v_sb[:])
                nc.sync.dma_start(out=dbg_OT[:], in_=OT[:])
            for s in range(N_CORES):
                nc.sync.dma_start(out=a2a_in[s], in_=OT[:, SH * s:SH * s + SH])
            nc.gpsimd.collective_compute(
                "AllToAll", Alu.bypass,
                replica_groups=[list(range(N_CORES))],
                ins=[a2a_in[:]], outs=[a2a_out[:]],
            )
            with tc.tile_pool(name="cproj", bufs=1) as cpp, \
                 tc.tile_pool(name="yps", bufs=4, space="PSUM") as ypsp:
                ot_recv = cpp.tile([128, N_CORES, SH], BF16, name="ot_recv")
                for s in range(N_CORES):
                    nc.sync.dma_start(out=ot_recv[:, s, :], in_=a2a_out[s])
                y_sb = cpp.tile([128, n_yrt, C], F32, name="y_sb")
                for rt in range(n_yrt):
                    for cb in range(2):
                        y_ps = ypsp.tile([128, 512], F32, name="y_ps")
                        for kt in range(N_CORES):
                            nc.tensor.matmul(
                                y_ps[:],
                                ot_recv[:, kt, 128 * rt:128 * rt + 128],
                                wc_sb[:, kt, 512 * cb:512 * cb + 512],
                                start=(kt == 0), stop=False)
                        nc.tensor.matmul(
                            y_ps[:], ones_bf[:],
                            bc_sb[:, 512 * cb:512 * cb + 512],
                            start=False, stop=True)
                        nc.any.tensor_copy(
                            y_sb[:, rt, 512 * cb:512 * cb + 512], y_ps[:])
                for rt in range(n_yrt):
                    nc.sync.dma_start(
                        out=y_ext[128 * rt:128 * rt + 128, :],
                        in_=y_sb[:, rt, :])

    nc.compile()
    return nc


def _host_prep(q_x, wq_f, bq_f, wk_f, bk_f, wv_f, bv_f, wc_f, bc_f, t):
    """Host-side: slice/cast per-core inputs, build tables."""
    rows = B * t
    bf = ml_dtypes.bfloat16
    xb = np.ascontiguousarray(np.asarray(q_x).reshape(rows, C)).astype(bf)

    # rope tables, matching reference._rope_tables
    inv_freq = 1.0 / (THETA ** (np.arange(0, HS, 2, dtype=np.float32) / HS))
    tt = np.arange(t, dtype=np.float32)
    freqs = tt[:, None] * inv_freq[None, :]          # [t, 32]
    emb = np.concatenate([freqs, freqs], axis=-1)    # [t, 64]
    cos = np.cos(emb).T.astype(np.float32)           # [64, t]
    sin = np.sin(emb).T.astype(np.float32)
    sin_signed = sin.copy()
    sin_signed[0:32] = -sin[0:32]
    cosT = np.tile(cos, (2, 1)).astype(bf)           # [128, t]
    sinT = np.tile(sin_signed, (2, 1)).astype(bf)

    tril = np.triu(np.ones((128, 128), np.float32)).astype(bf)

    def rot_bias(bslice):
        out = np.empty_like(bslice)
        for hh in range(2):
            out[64 * hh:64 * hh + 32] = bslice[64 * hh + 32:64 * hh + 64]
            out[64 * hh + 32:64 * hh + 64] = bslice[64 * hh:64 * hh + 32]
        return out

    wq_f = np.asarray(wq_f)
    wk_f = np.asarray(wk_f)
    wv_f = np.asarray(wv_f)
    wc_bf = np.asarray(wc_f).astype(bf)
    bc_bf = np.asarray(bc_f).astype(bf)

    in_maps = []
    for c in range(N_CORES):
        sl = slice(128 * c, 128 * c + 128)
        bq_c = np.asarray(bq_f)[sl].astype(np.float32)
        bk_c = np.asarray(bk_f)[sl].astype(np.float32)
        bv_c = np.asarray(bv_f)[sl].astype(np.float32)
        bvp = np.zeros(VW, np.float32)
        bvp[0:HS] = bv_c[0:HS]
        bvp[HS + 1:2 * HS + 1] = bv_c[HS:2 * HS]
        in_maps.append({
            "xb": xb,
            "wq": np.ascontiguousarray(wq_f[:, sl]).astype(bf),
            "wk": np.ascontiguousarray(wk_f[:, sl]).astype(bf),
            "wv": np.ascontiguousarray(wv_f[:, sl]).astype(bf),
            "wc": wc_bf, "bcb": bc_bf,
            "bq": bq_c, "bk": bk_c,
            "bqs": rot_bias(bq_c), "bks": rot_bias(bk_c),
            "bvp": bvp,
            "cosd": cosT, "sind": sinT, "trild": tril,
        })
    return in_maps


def _get_nc(t):
    if t not in _CACHE:
        _CACHE[t] = _build(t)
    return _CACHE[t]


def _make_in_maps(inputs, t):
    return _host_prep(
        inputs["q_x"], inputs["wq"], inputs["bq"], inputs["wk"], inputs["bk"],
        inputs["wv"], inputs["bv"], inputs["wc"], inputs["bc"], t)


def _assemble(results, t):
    rows = B * t
    sh = rows // N_CORES
    y = np.empty((rows, C), np.float32)
    for c in range(N_CORES):
        y[sh * c:sh * (c + 1)] = results[c]["y"]
    return y.reshape(B, t, C)


def _run(inputs, t=T, trace=False):
    from concourse.bass_utils import run_bass_kernel_spmd
    nc = _get_nc(t)
    in_maps = _make_in_maps(inputs, t)
    res = run_bass_kernel_spmd(nc, in_maps, list(range(N_CORES)), trace=trace)
    return _assemble(res.results, t), res


def kernel(**inputs):
    y, _ = _run(inputs, t=T, trace=False)
    return y
